# revision 10
# baseline (speedup 1.0000x reference)
"""Trainium2 Bass kernel for nn_Attention_63995012710903.

Math: the reference's mask is `scores*tril - 1e10*(1-triu)`, which makes the
softmax argument (pre /sqrt(64)):
    p <  q : scores - 1e10   -> exp underflows to exactly 0 in fp32
    p == q : scores
    p >  q : 0               -> exp = 1
So attention collapses to, per (batch, head), with e_q = exp(s_qq/8),
cnt_q = n-1-q, Z_q = e_q + cnt_q:
    z[q,:] = (e_q * v[q,:] + sum_{p>q} v[p,:]) / Z_q
           = ((e_q - 1) * v[q,:] + S_incl[q,:]) / Z_q,   S_incl[q] = sum_{p>=q} v[p]
Only the DIAGONAL of the score matrix and suffix sums of V are needed; both
O(n^2) attention matmuls disappear.  out = z_flat @ W_O^T.

Sharding (8 cores): core c -> batch b=c//4, head-pair g=c%4 (heads 2g, 2g+1).
Each core computes q,k,v for its 2 heads (f-block of 128 of z_flat's 512
features), z for that block, and the PARTIAL output  z_blk @ W_O[:,blk]^T
([2048, 512]).  The unshard sums the 4 partials per batch on host (W_O is
column-sharded, per the problem's sharding hint).

v3 schedule: fine-grained per-tile software pipeline tuned for the HAM clock
gate (PE idle => clock drops to 1.2GHz; ~3-10us of continuous matmul activity
to reach 2.4GHz):
    iter t (15..0): proj(t) | sfx(t+2) | WO(t+8)
z chunks 3..1 are transposed on the DMA XBAR (issued 3+ iterations before
their W_O consumers); chunk 0 is transposed on the PE (identity matmul) so
the drain stays matmul-dense instead of idling on the XBAR.  GpSimd cannot
touch PSUM, so it takes all SBUF-side elementwise work (R chain, t1,
softmax-denominator chain) while Scalar/Vector split the PSUM evacuations.
"""

import os
import sys

import numpy as np

for _p in ("/opt/trn_rl_repo", "/root/.axon_site/_ro/trn_rl_repo"):
    if os.path.isdir(_p) and _p not in sys.path:
        sys.path.insert(0, _p)

import ml_dtypes  # noqa: E402

import concourse.bass as bass  # noqa: E402
import concourse.tile as tile  # noqa: E402
from concourse import bacc, mybir  # noqa: E402
from concourse.bass_utils import run_bass_kernel_spmd  # noqa: E402


def _install_ntff_hook_shim():
    """antenv.axon_hooks is missing in this image, so the boot-time NTFF
    profile hook registration silently degraded.  Recreate the module and
    register the ctypes hook so trace=True yields exec_time_ns."""
    try:
        import antenv.axon_hooks  # noqa: F401
        return
    except ImportError:
        pass
    try:
        import types

        import antenv
        mod = types.ModuleType("antenv.axon_hooks")
        holder = {}
        mod.set_axon_ntff_profile_hook = lambda h: holder.__setitem__("h", h)
        mod.get_axon_ntff_profile_hook = lambda: holder.get("h")
        sys.modules["antenv.axon_hooks"] = mod
        antenv.axon_hooks = mod
        from trn_agent_boot.trn_boot import _ntff_profile_via_ctypes
        hook = _ntff_profile_via_ctypes("/opt/axon/libaxon_pjrt.so")
        if hook is not None:
            mod.set_axon_ntff_profile_hook(hook)
    except Exception:
        pass


_install_ntff_hook_shim()

BF16 = mybir.dt.bfloat16
F32 = mybir.dt.float32
NPBF16 = ml_dtypes.bfloat16

P = 128          # partitions / positions per tile
NT = 16          # seq tiles (2048 / 128)
SEQ = 2048
DMODEL = 512
NCORES = 8

LAG_SFX = 2      # sfx(t + LAG_SFX) emitted at iter t
LAG_WO = 8       # W_O matmul lag
PO_SPLIT = 288   # po columns 0:PO_SPLIT evac'd by scalar, rest by vector
N_WARMUP = 12


def _build_nc():
    nc = bacc.Bacc("TRN2", target_bir_lowering=False, debug=False,
                   num_devices=NCORES)

    # xq[q, pd, j, c] = x^T[128*j + pd, 512*q + c] — one 512KB DMA per
    # quarter with 4KB-contiguous per-partition runs (full DMA rate)
    xT = nc.dram_tensor("xT", [4, P, 4, DMODEL], BF16,
                        kind="ExternalInput").ap()
    wqkv = nc.dram_tensor("wqkv", [P, 4, 384], BF16, kind="ExternalInput").ap()
    wo = nc.dram_tensor("wo", [P, DMODEL], BF16, kind="ExternalInput").ap()
    trit = nc.dram_tensor("trit", [P, P], BF16, kind="ExternalInput").ap()
    ident = nc.dram_tensor("ident", [P, P], BF16, kind="ExternalInput").ap()
    cnt = nc.dram_tensor("cnt", [P, NT, 2], F32, kind="ExternalInput").ap()
    # p-major output: out[p, t, d] = row 128*t+p of the logical [2048, 512]
    # partial (keeps the store DMA's per-partition runs 4KB-contiguous)
    out = nc.dram_tensor("out", [P, NT, DMODEL], BF16,
                         kind="ExternalOutput").ap()

    with tile.TileContext(nc) as tc:
        _body(tc, out, xT, wqkv, wo, trit, ident, cnt)
    nc.compile()
    return nc


def _body(tc, out, xT, wqkv, wo, trit, ident, cnt):
    nc = tc.nc
    mult = mybir.AluOpType.mult
    add = mybir.AluOpType.add
    Exp = mybir.ActivationFunctionType.Exp

    with (
        tc.tile_pool(name="const", bufs=1) as const,
        tc.tile_pool(name="xpool", bufs=1) as xpool,
        tc.tile_pool(name="kvpool", bufs=3) as kvpool,
        tc.tile_pool(name="rpool", bufs=5) as rpool,
        tc.tile_pool(name="spool", bufs=1) as spool,
        tc.tile_pool(name="cscr", bufs=2) as cscr,
        tc.tile_pool(name="qkscr", bufs=2) as qkscr,
        tc.tile_pool(name="zcpool", bufs=5) as zcpool,
        tc.tile_pool(name="ztpool", bufs=3) as ztpool,
        tc.tile_pool(name="osb", bufs=3) as osb,
        tc.tile_pool(name="pproj", bufs=2, space="PSUM") as pproj,
        tc.tile_pool(name="psfx", bufs=2, space="PSUM") as psfx,
        tc.tile_pool(name="ptr", bufs=2, space="PSUM") as ptr,
        tc.tile_pool(name="pout", bufs=2, space="PSUM") as pout,
    ):
        # ---- PE warm-up: dummy matmuls during the DMA head so the HAM
        # clock gate ramps toward 2.4 GHz before real matmuls arrive ----
        wup = const.tile([P, DMODEL], BF16, name="wup")
        nc.vector.memset(wup[:], 0.0)
        pwup = pout.tile([P, DMODEL], F32, tag="po", name="pwup")
        for _ in range(N_WARMUP):
            nc.tensor.matmul(pwup[:], wup[:, 0:P], wup[:], start=True,
                             stop=True)

        # ---- weights (needed by first matmul), then x quarter-by-quarter
        # backward (processing runs backward from tile 15) ----
        # xsb[pd, q, j, c]: quarter q, d-chunk j, col c (within quarter)
        xsb = xpool.tile([P, 4, 4, DMODEL], BF16)
        nc.sync.dma_start(xsb[:, 3, :, :], xT[3])
        wsb = const.tile([P, 4, 384], BF16)
        nc.sync.dma_start(wsb[:], wqkv[:])

        trisb = const.tile([P, P], BF16)
        nc.gpsimd.dma_start(trisb[:], trit[:])
        idsb = const.tile([P, P], BF16)
        nc.gpsimd.dma_start(idsb[:], ident[:])
        cntsb = const.tile([P, NT, 2], F32)
        nc.gpsimd.dma_start(cntsb[:], cnt[:])
        wosb = const.tile([P, DMODEL], BF16)
        nc.gpsimd.dma_start(wosb[:], wo[:])
        onesb = const.tile([P, P], BF16)
        nc.vector.memset(onesb[:], 1.0)

        for q in (2, 1, 0):
            nc.sync.dma_start(xsb[:, q, :, :], xT[q])

        # persistent per-position scalars: [128, tile(16), head(2)]
        s_full = spool.tile([P, NT, 2], F32)   # diag scores
        w_full = spool.tile([P, NT, 2], F32)   # 1/Z
        a_full = spool.tile([P, NT, 2], F32)   # (e-1)/Z

        R = {}       # R[t] = sum of v tiles t..15 (bf16)
        kvcs = {}    # kvcs[c] = [128, 4(tile), 256(k|v)] bf16
        psx = {}     # psfx psum tiles
        zccs = {}    # combined z chunk tiles (bf16, [pos, 4, feat])
        ztcs = {}    # transposed z chunk tiles in SBUF ([feat, 4, pos])
        o4s = {}     # output half-chunk sbuf tiles
        t1cs = {}

        # ---------------- pipeline stages ----------------
        def proj(t):
            c, i = t // 4, t % 4
            pq = pproj.tile([P, 384], F32, tag="pq", name=f"pq_{t}")
            for dj in range(4):
                nc.tensor.matmul(
                    pq[:],
                    xsb[:, c, dj, P * i:P * (i + 1)],
                    wsb[:, dj, :],
                    start=(dj == 0),
                    stop=(dj == 3),
                )
            if i == 3:
                kvc = kvpool.tile([P, 4, 256], BF16, tag="kv", name=f"kv_{c}")
                kvcs[c] = kvc
            kvc = kvcs[c]
            # k|v evacuation (scalar)
            nc.scalar.copy(kvc[:, i, :], pq[:, 128:384])
            # diag q*k (vector) + per-head reduce (vector)
            qk = qkscr.tile([P, P], F32, tag="qk", name=f"qk_{t}")
            nc.vector.tensor_mul(qk[:], pq[:, 0:128], kvc[:, i, 0:128])
            nc.vector.tensor_reduce(
                s_full[:, t:t + 1, :],
                qk.rearrange("p (h f) -> p h f", h=2).unsqueeze(1),
                axis=mybir.AxisListType.X, op=add)
            # R chain (gpsimd, bf16, SBUF only)
            if t == NT - 1:
                R[t] = kvc[:, i, 128:256]
            else:
                r_new = rpool.tile([P, P], BF16, tag="r", name=f"r_{t}")
                nc.vector.tensor_add(r_new[:], kvc[:, i, 128:256], R[t + 1])
                R[t] = r_new[:]

        def chunk_scalars(c):
            t0 = 4 * c
            e_scr = cscr.tile([P, 4, 2], F32, tag="e", name=f"e_{c}")
            nc.scalar.activation(e_scr[:], s_full[:, t0:t0 + 4, :], Exp,
                                 scale=0.125)
            z_scr = cscr.tile([P, 4, 2], F32, tag="zz", name=f"zz_{c}")
            nc.vector.tensor_add(z_scr[:], e_scr[:], cntsb[:, t0:t0 + 4, :])
            nc.vector.reciprocal(w_full[:, t0:t0 + 4, :], z_scr[:])
            em1 = cscr.tile([P, 4, 2], F32, tag="em1", name=f"em1_{c}")
            nc.vector.tensor_scalar_add(em1[:], e_scr[:], -1.0)
            nc.vector.tensor_mul(a_full[:, t0:t0 + 4, :], em1[:],
                                 w_full[:, t0:t0 + 4, :])
            # t1 = a * v for the whole chunk (one op per head, gpsimd)
            t1c = cscr.tile([P, 4, 2, 64], F32, tag="t1", name=f"t1_{c}")
            kvc = kvcs[c]
            for h in (0, 1):
                in0 = kvc[:, :, 128 + 64 * h:128 + 64 * (h + 1)]
                in1 = a_full[:, t0:t0 + 4, h:h + 1]
                b0, b1 = bass.broadcast_tensor_aps(in0, in1)
                nc.vector.tensor_tensor(t1c[:, :, h, :], b0, b1, op=mult)
            t1cs[c] = t1c

        def sfx(t):
            # suffix-sum matmuls into psum: tri^T @ v(t) (+ ones^T @ R[t+1])
            c, i = t // 4, t % 4
            ps = psfx.tile([P, P], F32, tag="ps", name=f"ps_{t}")
            nc.tensor.matmul(ps[:], trisb[:], kvcs[c][:, i, 128:256],
                             start=True, stop=(t == NT - 1))
            if t < NT - 1:
                nc.tensor.matmul(ps[:], onesb[:], R[t + 1],
                                 start=False, stop=True)
            psx[t] = ps

        def stt(t):
            # zc = w * ps + t1  (per head; scalar operand is per-partition)
            c, i = t // 4, t % 4
            if i == 3:
                zcc = zcpool.tile([P, 4, P], BF16, tag="zc", name=f"zc_{c}")
                zccs[c] = zcc
            zcc = zccs[c]
            t1c = t1cs[c]
            for h in (0, 1):
                sl = slice(64 * h, 64 * (h + 1))
                nc.vector.scalar_tensor_tensor(
                    zcc[:, i, sl], psx[t][:, sl],
                    w_full[:, t, h:h + 1], t1c[:, i, h, :],
                    op0=mult, op1=add)
            # once a chunk's 4 z tiles are combined, ship it to the XBAR
            # (chunks 3..1; chunk 0 is PE-transposed in the drain)
            if i == 0 and c > 0:
                ztc = ztpool.tile([P, 4, P], BF16, tag="ztc", name=f"ztc_{c}")
                nc.sync.dma_start_transpose(ztc[:], zcc[:])
                ztcs[c] = ztc

        def wo_stage(t, zt_ap):
            hc, i = t // 2, t % 2   # half-chunk of 2 tiles
            po = pout.tile([P, DMODEL], F32, tag="po", name=f"po_{t}")
            nc.tensor.matmul(po[:], zt_ap, wosb[:], start=True, stop=True)
            if i == 1:
                o4 = osb.tile([P, 2, DMODEL], BF16, tag="o", name=f"o4_{hc}")
                o4s[hc] = o4
            o4 = o4s[hc]
            nc.scalar.copy(o4[:, i, 0:PO_SPLIT], po[:, 0:PO_SPLIT])
            nc.vector.tensor_copy(o4[:, i, PO_SPLIT:512], po[:, PO_SPLIT:512])
            if i == 0:
                nc.sync.dma_start(out[:, 2 * hc:2 * hc + 2, :], o4[:])

        # ---------------- main loop (backward over tiles) ----------------
        for t in reversed(range(NT)):
            proj(t)
            if t % 4 == 0:
                chunk_scalars(t // 4)
            if t + 3 < NT:
                stt(t + 3)
            if t + LAG_SFX < NT:
                sfx(t + LAG_SFX)
            if t + LAG_WO < NT:
                tw = t + LAG_WO
                wo_stage(tw, ztcs[tw // 4][:, tw % 4, :])

        # ---------------- drain ----------------
        sfx(1)
        sfx(0)
        for t in (2, 1, 0):
            stt(t)
        # chunk-1 W_O first (XBAR output ready) while vector finishes the
        # last combines; then chunk 0 transposed on the PE to keep the
        # drain matmul-dense
        wo_stage(7, ztcs[1][:, 3, :])
        wo_stage(6, ztcs[1][:, 2, :])
        zt0 = ztpool.tile([P, 4, P], BF16, tag="zt0", name="zt0", bufs=1)
        pT3 = ptr.tile([P, P], BF16, tag="pt", name="pt_3")
        nc.tensor.transpose(pT3[:], zccs[0][:, 3, :], idsb[:])
        nc.vector.tensor_copy(zt0[:, 3, :], pT3[:])
        wo_stage(5, ztcs[1][:, 1, :])
        wo_stage(4, ztcs[1][:, 0, :])
        for t in (2, 1, 0):
            pT = ptr.tile([P, P], BF16, tag="pt", name=f"pt_{t}")
            nc.tensor.transpose(pT[:], zccs[0][:, t, :], idsb[:])
            nc.vector.tensor_copy(zt0[:, t, :], pT[:])
        for t in (3, 2, 1, 0):
            wo_stage(t, zt0[:, t, :])


def _emit_noop():
    pass


_NC_CACHE = {}


def _get_nc():
    if "nc" not in _NC_CACHE:
        _NC_CACHE["nc"] = _build_nc()
    return _NC_CACHE["nc"]


def _make_in_maps(x, W_Q, W_K, W_V, W_O):
    tri = np.tril(np.ones((P, P), np.float32)).astype(NPBF16)
    idn = np.eye(P, dtype=np.float32).astype(NPBF16)
    pos = (np.arange(NT)[None, :] * P + np.arange(P)[:, None]).astype(np.float32)
    cnt1 = (SEQ - 1) - pos                      # [128, 16]
    cnt = np.stack([cnt1, cnt1], axis=2)        # [128, 16, 2]
    cnt = np.ascontiguousarray(cnt, np.float32)

    in_maps = []
    for core in range(NCORES):
        b, g = core // 4, core % 4
        xTb = np.asarray(x[b]).T.astype(NPBF16)          # [512, 2048]
        # -> [q, pd, j, c]: xq[q, pd, j, c] = xT[128j+pd, 512q+c]
        xq = np.ascontiguousarray(
            xTb.reshape(4, P, 4, DMODEL).transpose(2, 1, 0, 3))
        wq = np.asarray(W_Q[2 * g:2 * g + 2]).reshape(P, DMODEL).T
        wk = np.asarray(W_K[2 * g:2 * g + 2]).reshape(P, DMODEL).T
        wv = np.asarray(W_V[2 * g:2 * g + 2]).reshape(P, DMODEL).T
        wqkv = np.concatenate([wq, wk, wv], axis=1).astype(NPBF16)  # [512,384]
        # -> [pd, j, 384]: whost[pd, j, :] = wqkv[128j+pd, :]
        whost = np.ascontiguousarray(
            wqkv.reshape(4, P, 384).transpose(1, 0, 2))
        wo_c = np.ascontiguousarray(
            np.asarray(W_O)[:, P * g:P * (g + 1)].T).astype(NPBF16)
        in_maps.append({
            "xT": xq, "wqkv": whost, "wo": wo_c,
            "trit": tri, "ident": idn, "cnt": cnt,
        })
    return in_maps


def _run(x, W_Q, W_K, W_V, W_O, trace=False, **spmd_kwargs):
    nc = _get_nc()
    in_maps = _make_in_maps(x, W_Q, W_K, W_V, W_O)
    res = run_bass_kernel_spmd(nc, in_maps, core_ids=list(range(NCORES)),
                               trace=trace, **spmd_kwargs)
    # device output is p-major [128, 16, 512]; back to [2048, 512]
    outs = [r["out"].astype(np.float32).transpose(1, 0, 2).reshape(SEQ, DMODEL)
            for r in res.results]
    full = np.stack([
        outs[0] + outs[1] + outs[2] + outs[3],
        outs[4] + outs[5] + outs[6] + outs[7],
    ])  # [2, 2048, 512]
    return full, res


def kernel(x, W_Q, W_K, W_V, W_O):
    full, _ = _run(np.asarray(x), np.asarray(W_Q), np.asarray(W_K),
                   np.asarray(W_V), np.asarray(W_O))
    return full
